# revision 21
# baseline (speedup 1.0000x reference)
# Bass/Tile kernel for nn_Decoder: 4-layer dense transformer, B=2 L=2048 D=1024 H=16 V=32000.
# Sharding: token-parallel over 8 cores (core c owns the 512-token quarter c%4 of batch c//4),
# per-layer split K/V AllGathers (bf16) within batch groups [[0-3],[4-7]], token-sharded
# logits (no collective; each core computes its own tokens over the full vocab).
# Layout: activations transposed xT [D(part-tiled), T] fp32 residual; all matmul operands
# bf16 (weights host-pretiled/contiguous); PSUM fp32; causality via per-core 0/1 bf16 mask
# data resident in SBUF (program identical across cores - SPMD).
import contextlib
import numpy as np
import concourse.bass as bass
import concourse.mybir as mybir
import concourse.tile as tile
from concourse import bacc
from concourse.masks import make_identity

P = 128
D = 1024
H = 16
DH = 64
FF = 2048
L = 2048
B = 2
V = 32000
NCORE = 8
T = 512            # own tokens per core
KO = D // P        # 8
FKO = FF // P      # 16
NKT = (4 * T) // P  # 16 key tiles (full batch)
VN = 500           # vocab N-tile width
NVT = V // VN      # 64
EPS = 1e-6
f32 = mybir.dt.float32
bf16 = mybir.dt.bfloat16
AF = mybir.ActivationFunctionType
ALU = mybir.AluOpType
KB = D * T          # elements of kT per core
VB = T * H * (DH + 1)  # elements of v-rows (with ones column) per core


def build(n_layers=4, do_logits=True, has_bias=False, nonzero_bv=False,
          nonzero_blog=False, want_xout=False):
    nc = bacc.Bacc(None, target_bir_lowering=False)

    x0T = nc.dram_tensor("x0T", [D, T], f32, kind="ExternalInput")
    wq = nc.dram_tensor("wq", [n_layers, KO, P, KO, P], bf16, kind="ExternalInput")
    wk = nc.dram_tensor("wk", [n_layers, KO, P, KO, P], bf16, kind="ExternalInput")
    wv = nc.dram_tensor("wv", [n_layers, KO, P, KO, P], bf16, kind="ExternalInput")
    wo = nc.dram_tensor("wo", [n_layers, KO, P, KO, P], bf16, kind="ExternalInput")
    wproj = nc.dram_tensor("wproj", [n_layers, FKO, P, KO, P], bf16, kind="ExternalInput")
    wup = nc.dram_tensor("wup", [n_layers, FKO, P, KO, P], bf16, kind="ExternalInput")
    wdown = nc.dram_tensor("wdown", [n_layers, KO, P, FKO, P], bf16, kind="ExternalInput")
    maskbig = nc.dram_tensor("maskbig", [NKT, P, T], bf16, kind="ExternalInput")
    # bias columns: [bq 0:8 | bk 8:16 | bv 16:24 | bo 24:32 | bproj 32:48 | bup 48:64
    #                | bdown 64:72]
    ball = (nc.dram_tensor("ball", [n_layers, P, 72], f32, kind="ExternalInput")
            if has_bias else None)
    if do_logits:
        wlog = nc.dram_tensor("wlog", [NVT, P, KO, VN], bf16, kind="ExternalInput")
        blogmat = (nc.dram_tensor("blogmat", [P, VN * NVT], bf16, kind="ExternalInput")
                   if nonzero_blog else None)
        e0_d = (nc.dram_tensor("e0_d", [P, P], bf16, kind="ExternalInput")
                if nonzero_blog else None)
        logits = nc.dram_tensor("logits", [T, V], bf16, kind="ExternalOutput")
    if want_xout:
        xout = nc.dram_tensor("xout", [P, KO, T], f32, kind="ExternalOutput")

    with tile.TileContext(nc) as tc, contextlib.ExitStack() as ctx:
        cn = ctx.enter_context(tc.tile_pool(name="cn", bufs=1))
        pb = ctx.enter_context(tc.tile_pool(name="pb", bufs=1))
        nrm = ctx.enter_context(tc.tile_pool(name="nrm", bufs=2))   # normT double buf
        evn = ctx.enter_context(tc.tile_pool(name="evn", bufs=2))   # norm/small temps
        eva = ctx.enter_context(tc.tile_pool(name="eva", bufs=2))   # evict stream temps
        ps = ctx.enter_context(tc.tile_pool(name="ps", bufs=2, space="PSUM"))
        dr = ctx.enter_context(tc.tile_pool(name="dram", bufs=1, space="DRAM"))

        # ---- constants ----
        ones_bf = cn.tile([P, P], bf16, tag="ones_bf")
        nc.any.memset(ones_bf[:], 1.0)
        ident_bf = cn.tile([P, P], bf16, tag="ident_bf")
        make_identity(nc, ident_bf)
        sc_rms = cn.tile([P, 1], f32, tag="sc_rms")
        nc.any.memset(sc_rms[:], 1.0 / D)
        eps_t = cn.tile([P, 1], f32, tag="eps")
        nc.any.memset(eps_t[:], EPS)
        sc_att = cn.tile([P, 1], f32, tag="sc_att")
        nc.any.memset(sc_att[:], 0.125)
        e0 = None
        if do_logits and nonzero_blog:
            e0 = cn.tile([P, P], bf16, tag="e0")
            nc.sync.dma_start(e0[:], e0_d[:])

        # ---- persistent activations ----
        xT = pb.tile([P, KO, T], f32, tag="xT")
        QT = pb.tile([P, KO, T], bf16, tag="QT")
        aoT = pb.tile([P, KO, T], bf16, tag="aoT")
        mask_sb = pb.tile([P, NKT, T], bf16, tag="mask")
        nc.sync.dma_start(mask_sb[:], maskbig.rearrange("k p t -> p k t"))

        # ---- load x ----
        nc.sync.dma_start(xT[:], x0T.rearrange("(ko p) t -> p ko t", p=P))

        def rmsnorm(nm):
            normT = nrm.tile([P, KO, T], bf16, tag="nta", name=nm)
            ssq = ps.tile([P, T], f32, tag="mm512")
            for ko in range(KO):
                x2 = evn.tile([P, T], bf16, tag="x2")
                nc.vector.tensor_tensor(x2[:], xT[:, ko], xT[:, ko], ALU.mult)
                nc.tensor.matmul(ssq[:], ones_bf[:], x2[:], start=(ko == 0),
                                 stop=(ko == KO - 1))
            srt = evn.tile([P, T], f32, tag="srt")
            nc.scalar.activation(srt[:], ssq[:], AF.Sqrt, bias=eps_t[:], scale=sc_rms[:])
            inv = evn.tile([P, T], f32, tag="inv")
            nc.vector.reciprocal(inv[:], srt[:])
            nc.vector.tensor_tensor(
                normT[:], xT[:], inv[:, None, :].to_broadcast([P, KO, T]), ALU.mult)
            return normT

        with tc.tile_pool(name="wp", bufs=2) as wp, \
             tc.tile_pool(name="kvr", bufs=1) as kvr, \
             tc.tile_pool(name="exp", bufs=3) as exp_, \
             tc.tile_pool(name="pa", bufs=2, space="PSUM") as pa, \
             tc.tile_pool(name="pv", bufs=2, space="PSUM") as pv, \
             tc.tile_pool(name="acp", bufs=1) as acp:

            def proj_T(w_, i, src, consume, bias_col=None, n_mt=KO, n_ko=KO):
                for m in range(n_mt):
                    wt = wp.tile([P, n_ko, P], bf16, tag="wlhs")
                    nc.sync.dma_start(wt[:], w_[i, m])
                    pt = ps.tile([P, T], f32, tag="mm512")
                    for ko in range(n_ko):
                        nc.tensor.matmul(pt[:], wt[:, ko], src[:, ko], start=(ko == 0),
                                         stop=(ko == n_ko - 1))
                    consume(m, pt, bias_col[:, m:m + 1] if bias_col is not None else None)

            for li in range(n_layers):
                bias_t = None
                if has_bias:
                    bias_t = evn.tile([P, 72], f32, tag="bias")
                    nc.sync.dma_start(bias_t[:], ball[li])

                normT = rmsnorm(f"norm1_{li}")

                kag_in = dr.tile([KB], bf16, tag="kag_in")
                kag_out = dr.tile([4, KB], bf16, tag="kag_out")
                vag_in = dr.tile([VB], bf16, tag="vag_in")
                vag_out = dr.tile([4, VB], bf16, tag="vag_out")
                kT_view = kag_in.rearrange("(d t) -> d t", t=T)
                vrow_view = vag_in.rearrange("(t h e) -> t h e", h=H, e=DH + 1)

                # K/V SBUF residents: block 0 = own tokens (written directly by the
                # projections, usable before the AllGather); block j = rank (q+j)%4
                # (dynamic-offset DMA from the gather output).
                K_sb = kvr.tile([P, KO, 4 * T], bf16, tag="K_sb", name=f"K_{li}")
                V_sb = kvr.tile([P, NKT, H, DH + 1], bf16, tag="V_sb", name=f"V_{li}")

                def k_consume(m, pt, bcol, kT_view=kT_view, K_sb=K_sb):
                    if bcol is not None:
                        nc.scalar.activation(K_sb[:, m, 0:T], pt[:], AF.Identity,
                                             bias=bcol[:])
                    else:
                        nc.scalar.activation(K_sb[:, m, 0:T], pt[:], AF.Copy)
                    nc.sync.dma_start(kT_view[m * P:(m + 1) * P, :], K_sb[:, m, 0:T])
                proj_T(wk, li, normT, k_consume,
                       bias_col=bias_t[:, 8:16] if has_bias else None)
                nc.gpsimd.collective_compute(
                    "AllGather", ALU.bypass, ins=[kag_in[:]], outs=[kag_out[:]],
                    replica_groups=[[0, 1, 2, 3], [4, 5, 6, 7]])

                # V: compute vT like K, then PE-transpose into row-major [tok, h, dh+1]
                vT_sb = kvr.tile([P, KO, T], bf16, tag="vT", name=f"vT_{li}")

                def v_consume(m, pt, bcol, vT_sb=vT_sb):
                    if bcol is not None:
                        nc.scalar.activation(vT_sb[:, m, :], pt[:], AF.Identity,
                                             bias=bcol[:])
                    else:
                        nc.scalar.activation(vT_sb[:, m, :], pt[:], AF.Copy)
                proj_T(wv, li, normT, v_consume,
                       bias_col=bias_t[:, 16:24] if has_bias else None)
                for tt in range(T // P):
                    for ko in range(KO):
                        tp = pv.tile([P, P], bf16, tag="vaux")
                        nc.tensor.transpose(tp[:], vT_sb[:, ko, tt * P:(tt + 1) * P],
                                            ident_bf[:])
                        nc.scalar.activation(
                            V_sb[:, tt, 2 * ko:2 * ko + 2, 0:DH],
                            tp[:].rearrange("p (a b) -> p a b", a=2), AF.Copy)
                    nc.vector.tensor_copy(V_sb[:, tt, :, DH:DH + 1],
                                          ones_bf[:, 0:H, None])
                    nc.sync.dma_start(vrow_view[tt * P:(tt + 1) * P], V_sb[:, tt])
                nc.gpsimd.collective_compute(
                    "AllGather", ALU.bypass, ins=[vag_in[:]], outs=[vag_out[:]],
                    replica_groups=[[0, 1, 2, 3], [4, 5, 6, 7]])

                def q_consume(m, pt, bcol):
                    if bcol is not None:
                        nc.scalar.activation(QT[:, m, :], pt[:], AF.Identity, bias=bcol[:])
                    else:
                        nc.scalar.activation(QT[:, m, :], pt[:], AF.Copy)
                proj_T(wq, li, normT, q_consume,
                       bias_col=bias_t[:, 0:8] if has_bias else None)

                # remote K/V blocks: rank (pid + j) % 4 of the gather output
                pid = nc.sync.partition_id()
                for j in range(1, 4):
                    rj = (pid + j) % 4
                    nc.sync.dma_start(
                        K_sb[:, :, j * T:(j + 1) * T],
                        kag_out[bass.ds(rj, 1)].rearrange(
                            "a (ko p t) -> p (a ko) t", p=P, t=T))
                    nc.sync.dma_start(
                        V_sb[:, 4 * j:4 * (j + 1)],
                        vag_out[bass.ds(rj, 1)].rearrange(
                            "a (tt p h e) -> p (a tt) h e", p=P, h=H, e=DH + 1))
                for hp in range(H // 2):
                    avps = [pa.tile([DH + 1, T], f32, tag="avp", name=f"avp{hp}_{s}")
                            for s in range(2)]
                    for kt in range(NKT):
                        for s in range(2):
                            pbase = DH * s
                            sp = ps.tile([P, T], f32, tag="mm512")
                            nc.tensor.matmul(sp[:],
                                             K_sb[pbase:pbase + DH, hp,
                                                  kt * P:(kt + 1) * P],
                                             QT[pbase:pbase + DH, hp, :],
                                             start=True, stop=True)
                            ext = exp_.tile([P, T], bf16, tag="exs")
                            nc.scalar.activation(ext[:], sp[:], AF.Exp, scale=sc_att[:])
                            nc.vector.tensor_tensor(ext[:], ext[:], mask_sb[:, kt, :],
                                                    ALU.mult)
                            nc.tensor.matmul(avps[s][:], V_sb[:, kt, 2 * hp + s, :],
                                             ext[:],
                                             start=(kt == 0), stop=(kt == NKT - 1))
                    for s in range(2):
                        pbase = DH * s
                        avp = avps[s]
                        invd = evn.tile([DH + 1, T], f32, tag="invd")
                        nc.vector.reciprocal(invd[DH:DH + 1, :], avp[DH:DH + 1, :])
                        invdr = evn.tile([DH + 1, T], bf16, tag="invdr")
                        nc.vector.tensor_copy(invdr[DH:DH + 1, :], invd[DH:DH + 1, :])
                        bcp = pv.tile([DH, T], f32, tag="vaux")
                        nc.tensor.matmul(bcp[:], ones_bf[DH:DH + 1, 0:DH],
                                         invdr[DH:DH + 1, :], start=True, stop=True)
                        invb = evn.tile([DH, T], f32, tag="invb")
                        nc.scalar.activation(invb[:], bcp[:], AF.Copy)
                        nc.vector.tensor_tensor(aoT[pbase:pbase + DH, hp, :],
                                                avp[0:DH, :], invb[:], ALU.mult)

                # ---- O projection + residual (direct PSUM->residual add) ----
                def o_consume(m, pt, bcol):
                    if bcol is not None:
                        ot = eva.tile([P, T], f32, tag="evaf")
                        nc.scalar.activation(ot[:], pt[:], AF.Identity, bias=bcol[:])
                        nc.vector.tensor_tensor(xT[:, m, :], xT[:, m, :], ot[:], ALU.add)
                    else:
                        nc.vector.tensor_tensor(xT[:, m, :], pt[:], xT[:, m, :], ALU.add)
                proj_T(wo, li, aoT, o_consume,
                       bias_col=bias_t[:, 24:32] if has_bias else None)

                # ---- FFN ----
                normT = rmsnorm(f"norm2_{li}")
                hts = []
                for m in range(FKO):
                    wtp = wp.tile([P, KO, P], bf16, tag="wlhs")
                    nc.sync.dma_start(wtp[:], wproj[li, m])
                    ptp = ps.tile([P, T], f32, tag="mm512")
                    for ko in range(KO):
                        nc.tensor.matmul(ptp[:], wtp[:, ko], normT[:, ko],
                                         start=(ko == 0), stop=(ko == KO - 1))
                    wtu = wp.tile([P, KO, P], bf16, tag="wlhs2")
                    nc.sync.dma_start(wtu[:], wup[li, m])
                    ptu = ps.tile([P, T], f32, tag="mm512")
                    for ko in range(KO):
                        nc.tensor.matmul(ptu[:], wtu[:, ko], normT[:, ko],
                                         start=(ko == 0), stop=(ko == KO - 1))
                    usb = evn.tile([P, T], f32, tag="uev")
                    if has_bias:
                        nc.scalar.activation(usb[:], ptu[:], AF.Identity,
                                             bias=bias_t[:, 48 + m:49 + m])
                        psb = evn.tile([P, T], f32, tag="pev")
                        nc.scalar.activation(psb[:], ptp[:], AF.Identity,
                                             bias=bias_t[:, 32 + m:33 + m])
                        gsb = evn.tile([P, T], f32, tag="gev")
                        nc.vector.tensor_tensor(gsb[:], psb[:], usb[:], ALU.mult)
                    else:
                        nc.scalar.activation(usb[:], ptu[:], AF.Copy)
                        gsb = evn.tile([P, T], f32, tag="gev")
                        nc.vector.tensor_tensor(gsb[:], ptp[:], usb[:], ALU.mult)
                    ht = acp.tile([P, T], bf16, tag=f"acc{m}", name=f"ht{m}")
                    nc.scalar.activation(ht[:], gsb[:], AF.Silu)
                    hts.append(ht)
                for m in range(KO):
                    wt = wp.tile([P, FKO, P], bf16, tag="wdn")
                    nc.sync.dma_start(wt[:], wdown[li, m])
                    pt = ps.tile([P, T], f32, tag="mm512")
                    for ko in range(FKO):
                        nc.tensor.matmul(pt[:], wt[:, ko], hts[ko][:], start=(ko == 0),
                                         stop=(ko == FKO - 1))
                    if has_bias:
                        dt_ = eva.tile([P, T], f32, tag="evaf")
                        nc.scalar.activation(dt_[:], pt[:], AF.Identity,
                                             bias=bias_t[:, 64 + m:65 + m])
                        nc.vector.tensor_tensor(xT[:, m, :], xT[:, m, :], dt_[:], ALU.add)
                    else:
                        nc.vector.tensor_tensor(xT[:, m, :], pt[:], xT[:, m, :], ALU.add)

        if want_xout:
            nc.sync.dma_start(xout[:], xT[:])

        if do_logits:
            xr = pb.tile([P, KO, T], bf16, tag="QT", name="xr")
            nc.vector.tensor_copy(xr[:], xT[:])
            with tc.tile_pool(name="wlp", bufs=3) as wlp:
                for nt in range(NVT):
                    wt = wlp.tile([P, KO, VN], bf16, tag="wlog")
                    nc.sync.dma_start(wt[:], wlog[nt])
                    blt = None
                    if blogmat is not None:
                        blt = wlp.tile([P, VN], bf16, tag="wlogb")
                        nc.sync.dma_start(blt[:], blogmat[:, nt * VN:(nt + 1) * VN])
                    for mt in range(T // P):
                        pt = ps.tile([P, VN], f32, tag="mm512")
                        for ko in range(KO):
                            last = (ko == KO - 1) and blt is None
                            nc.tensor.matmul(pt[:], xr[:, ko, mt * P:(mt + 1) * P],
                                             wt[:, ko], start=(ko == 0), stop=last)
                        if blt is not None:
                            nc.tensor.matmul(pt[:], e0[:, :P], blt[:],
                                             start=False, stop=True)
                        lo = eva.tile([P, VN], bf16, tag="eva")
                        nc.scalar.activation(lo[:], pt[:], AF.Copy)
                        nc.sync.dma_start(
                            logits[mt * P:(mt + 1) * P, nt * VN:(nt + 1) * VN], lo[:])

    nc.compile()
    return nc


def _tile_lhs(w, n_mt):
    """[D_in, M] -> [n_mt, P, D_in//P, P] tiled-lhsT layout, bf16."""
    import ml_dtypes
    d_in, m_dim = w.shape
    n_ko = d_in // P
    # out[m, p, ko, mm] = w[ko*P+p, m*P+mm]
    wt = w.reshape(n_ko, P, n_mt, P).transpose(2, 1, 0, 3)
    return np.ascontiguousarray(wt.astype(ml_dtypes.bfloat16))


def host_inputs(inp, n_layers=4, do_logits=True):
    """Build per-core in_maps from the full model inputs dict (numpy)."""
    import math
    import ml_dtypes
    g = {k: np.asarray(v) for k, v in inp.items()}
    ids = g["input_ids"].astype(np.int64)
    embed = g["embed"].astype(np.float32)
    pos = np.arange(L, dtype=np.float32)[:, None]
    div = np.exp(np.arange(0, D, 2, dtype=np.float32) * (-math.log(10000.0) / D))
    ang = pos * div
    pe = np.zeros((L, D), dtype=np.float32)
    pe[:, 0::2] = np.sin(ang)
    pe[:, 1::2] = np.cos(ang)

    gam = g["gammas"].astype(np.float32)
    wq_ = np.stack([_tile_lhs(gam[2 * i][:, None] * g["Wq"][i], KO)
                    for i in range(n_layers)])
    wk_ = np.stack([_tile_lhs(gam[2 * i][:, None] * g["Wk"][i], KO)
                    for i in range(n_layers)])
    wv_ = np.stack([_tile_lhs(gam[2 * i][:, None] * g["Wv"][i], KO)
                    for i in range(n_layers)])
    wo_ = np.stack([_tile_lhs(g["Wo"][i], KO) for i in range(n_layers)])
    wp_ = np.stack([_tile_lhs(gam[2 * i + 1][:, None] * g["Wproj"][i], FKO)
                    for i in range(n_layers)])
    wu_ = np.stack([_tile_lhs(gam[2 * i + 1][:, None] * g["Wup"][i], FKO)
                    for i in range(n_layers)])
    wd_ = np.stack([_tile_lhs(g["Wdown"][i], KO) for i in range(n_layers)])

    has_bias = bool(
        np.any(g["bq"][:n_layers]) or np.any(g["bk"][:n_layers])
        or np.any(g["bv"][:n_layers]) or np.any(g["bo"][:n_layers])
        or np.any(g["bproj"][:n_layers]) or np.any(g["bup"][:n_layers])
        or np.any(g["bdown"][:n_layers]))
    ball = None
    if has_bias:
        ball = np.zeros((n_layers, P, 72), np.float32)
        for i in range(n_layers):
            ball[i, :, 0:8] = g["bq"][i].reshape(KO, P).T
            ball[i, :, 8:16] = g["bk"][i].reshape(KO, P).T
            ball[i, :, 16:24] = g["bv"][i].reshape(KO, P).T
            ball[i, :, 24:32] = g["bo"][i].reshape(KO, P).T
            ball[i, :, 32:48] = g["bproj"][i].reshape(FKO, P).T
            ball[i, :, 48:64] = g["bup"][i].reshape(FKO, P).T
            ball[i, :, 64:72] = g["bdown"][i].reshape(KO, P).T

    nonzero_blog = do_logits and bool(np.any(g["blogits"] != 0))
    wlog_t = None
    blogmat = None
    e0 = None
    if do_logits:
        wl = g["Wlogits"].astype(np.float32)  # [D, V]
        # out[nt, p, ko, v] = wl[ko*P+p, nt*VN+v]
        wlog_t = np.ascontiguousarray(
            wl.reshape(KO, P, NVT, VN).transpose(2, 1, 0, 3).astype(ml_dtypes.bfloat16))
        if nonzero_blog:
            blogmat = np.zeros((P, V), np.float32)
            blogmat[0, :] = g["blogits"]
            blogmat = blogmat.astype(ml_dtypes.bfloat16)
            e0 = np.zeros((P, P), np.float32)
            e0[0, :] = 1.0
            e0 = e0.astype(ml_dtypes.bfloat16)

    in_maps = []
    for c in range(NCORE):
        b, q = c // 4, c % 4
        tok = ids[b, q * T:(q + 1) * T]
        x0T_ = np.ascontiguousarray(
            (embed[tok] + pe[q * T:(q + 1) * T, :]).T, np.float32)
        i_idx = np.arange(P)[:, None]
        j_idx = np.arange(T)[None, :]
        mb = np.zeros((NKT, P, T), np.float32)
        for kt in range(NKT):
            g = (q + kt // 4) % 4
            mb[kt] = ((g * T + (kt % 4) * P + i_idx)
                      <= (q * T + j_idx)).astype(np.float32)
        m = {
            "x0T": x0T_,
            "wq": wq_, "wk": wk_, "wv": wv_, "wo": wo_,
            "wproj": wp_, "wup": wu_, "wdown": wd_,
            "maskbig": mb.astype(ml_dtypes.bfloat16),
        }
        if has_bias:
            m["ball"] = ball
        if do_logits:
            m["wlog"] = wlog_t
            if nonzero_blog:
                m["blogmat"] = blogmat
                m["e0_d"] = e0
        in_maps.append(m)
    return in_maps, has_bias, nonzero_blog


_CACHE = {}


def _get_nc(key):
    if key not in _CACHE:
        _CACHE[key] = build(n_layers=4, do_logits=True,
                            has_bias=key[0], nonzero_blog=key[1])
    return _CACHE[key]


def kernel(**inputs):
    """Full-model entry: takes setup_inputs() arrays, returns [B, L, V] float32 logits."""
    from concourse.bass_utils import run_bass_kernel_spmd
    in_maps, hb, nzbl = host_inputs(inputs, n_layers=4, do_logits=True)
    nc = _get_nc((hb, nzbl))
    res = run_bass_kernel_spmd(nc, in_maps, core_ids=list(range(NCORE)))
    out = np.empty((B, L, V), dtype=np.float32)
    for c in range(NCORE):
        b, q = c // 4, c % 4
        out[b, q * T:(q + 1) * T, :] = np.asarray(
            res.results[c]["logits"], dtype=np.float32)
    return out
